# revision 2
# baseline (speedup 1.0000x reference)
"""Trainium2 Bass kernel for MemorySpatialAttention.

Math (per batch b):
  f = LeakyReLU_0.1(BN(conv(x)))  with conv = full-length dot -> x[N,L] @ W[L,H]
  sim = f_in @ f_mem^T  banded to |i-j| <= 8 (17 neighbors, clamped at edges)
  attn = softmax_band(sim);  out = 0.5*x + 0.5*(attn @ mem)

Sharding: data-parallel over batch B=8 -> one batch per NeuronCore, no
collectives. Per core everything is computed in transposed layout [L|H, N]
so the L=56 / H=128 contractions sit on SBUF partitions.
"""
import sys
sys.path.insert(0, '/opt/trn_rl_repo')

import numpy as np

B, N, C, L, H = 8, 2048, 1, 56, 128
NB, HALF = 17, 8
T = N // 128          # 16 row tiles
W = 128 + 2 * HALF    # 144-wide band window per row tile
RATE = 0.5
BN_EPS = 1e-5
NEG_SLOPE = 0.1

_cache = {}


def _build_program():
    import concourse.bacc as bacc
    import concourse.tile as tile
    from concourse import mybir
    from concourse.masks import make_identity

    F32 = mybir.dt.float32
    AF = mybir.ActivationFunctionType
    AX = mybir.AxisListType

    nc = bacc.Bacc("TRN2", target_bir_lowering=False, debug=False)

    xT = nc.dram_tensor("xT", [L, N], F32, kind="ExternalInput")
    memT = nc.dram_tensor("memT", [L, N], F32, kind="ExternalInput")
    memN = nc.dram_tensor("memN", [N, L], F32, kind="ExternalInput")   # 0.5 * mem
    wT = nc.dram_tensor("wT", [L, H], F32, kind="ExternalInput")
    scb = nc.dram_tensor("scb", [H, 2], F32, kind="ExternalInput")     # BN scale, bias
    maskb = nc.dram_tensor("maskb", [3, 128, W], F32, kind="ExternalInput")
    outT = nc.dram_tensor("outT", [L, N], F32, kind="ExternalOutput")

    with tile.TileContext(nc) as tc:
        with tc.tile_pool(name="consts", bufs=1) as consts, \
             tc.tile_pool(name="work", bufs=3) as work, \
             tc.tile_pool(name="stats", bufs=4) as stats, \
             tc.tile_pool(name="pbig", bufs=3, space="PSUM") as pbig, \
             tc.tile_pool(name="pt", bufs=2, space="PSUM") as pt_pool, \
             tc.tile_pool(name="pc", bufs=3, space="PSUM") as pc_pool:

            xT_s = consts.tile([L, N], F32)
            memT_s = consts.tile([L, N], F32)
            wT_s = consts.tile([L, H], F32)
            scb_s = consts.tile([H, 2], F32)
            maskb_s = consts.tile([128, 3, W], F32)
            ident = consts.tile([128, 128], F32)
            fiT = consts.tile([H, N], F32)
            fmT = consts.tile([H, N], F32)

            nc.sync.dma_start(out=wT_s, in_=wT.ap())
            nc.sync.dma_start(out=scb_s, in_=scb.ap())
            nc.sync.dma_start(out=maskb_s, in_=maskb.ap().rearrange("t p w -> p t w"))
            nc.sync.dma_start(out=xT_s, in_=xT.ap())
            nc.sync.dma_start(out=memT_s, in_=memT.ap())
            make_identity(nc, ident)

            # ---- features: f^T = PRelu(BN(W^T @ x^T)) in [H, N] layout ----
            CH = 512
            for src, dst in ((xT_s, fiT), (memT_s, fmT)):
                for c in range(N // CH):
                    psF = pbig.tile([128, CH], F32, tag="pbig")
                    nc.tensor.matmul(psF, lhsT=wT_s, rhs=src[:, c * CH:(c + 1) * CH],
                                     start=True, stop=True)
                    nc.scalar.activation(dst[:, c * CH:(c + 1) * CH], psF, AF.Prelu,
                                         bias=scb_s[:, 1:2], scale=scb_s[:, 0:1],
                                         alpha=NEG_SLOPE)

            # ---- banded attention, one 128-row tile at a time ----
            for t in range(T):
                j0 = min(max(128 * t - HALF, 0), N - W)
                mtype = 0 if t == 0 else (2 if t == T - 1 else 1)

                psA = pbig.tile([128, W], F32, tag="pbig")
                nc.tensor.matmul(psA, lhsT=fiT[:, 128 * t:128 * (t + 1)],
                                 rhs=fmT[:, j0:j0 + W], start=True, stop=True)

                simS = work.tile([128, W], F32)
                nc.vector.tensor_add(simS, psA, maskb_s[:, mtype, :])
                negmax = stats.tile([128, 1], F32)
                nc.vector.reduce_max(negmax, simS, axis=AX.X, negate=True)

                E = work.tile([128, W], F32)
                rowsum = stats.tile([128, 1], F32)
                nc.scalar.activation(E, simS, AF.Exp, bias=negmax, scale=1.0,
                                     accum_out=rowsum)
                rinv = stats.tile([128, 1], F32)
                nc.vector.reciprocal(rinv, rowsum)
                att = work.tile([128, W], F32)
                nc.scalar.mul(att, E, rinv)

                psT = pt_pool.tile([128, 256], F32)
                nc.tensor.transpose(psT[:, 0:128], att[:, 0:128], ident)
                nc.tensor.transpose(psT[0:16, 128:256], att[:, 128:W], ident)
                attnT = work.tile([128, 256], F32)
                nc.vector.tensor_copy(attnT, psT)

                ms1 = work.tile([128, L], F32)
                ms2 = work.tile([16, L], F32)
                nc.sync.dma_start(out=ms1, in_=memN.ap()[j0:j0 + 128, :])
                nc.sync.dma_start(out=ms2, in_=memN.ap()[j0 + 128:j0 + W, :])

                psC = pc_pool.tile([L, 128], F32)
                nc.tensor.matmul(psC, lhsT=ms1, rhs=attnT[:, 0:128],
                                 start=True, stop=False)
                nc.tensor.matmul(psC, lhsT=ms2, rhs=attnT[0:16, 128:256],
                                 start=False, stop=True)

                xh = work.tile([L, 128], F32)
                nc.gpsimd.tensor_scalar_mul(xh, xT_s[:, 128 * t:128 * (t + 1)], RATE)
                ot = work.tile([L, 128], F32)
                nc.vector.tensor_add(ot, psC, xh)
                nc.sync.dma_start(out=outT.ap()[:, 128 * t:128 * (t + 1)], in_=ot)

    nc.compile()
    return nc


def _host_prep(input, state_memory, conv_w, conv_b, bn_gamma, bn_beta, bn_mean, bn_var):
    s = (bn_gamma / np.sqrt(bn_var + BN_EPS)).astype(np.float32)
    bias_h = ((conv_b - bn_mean) * s + bn_beta).astype(np.float32)
    scb = np.ascontiguousarray(np.stack([s, bias_h], axis=1))          # [H, 2]
    wT = np.ascontiguousarray(conv_w[:, 0, :].T).astype(np.float32)    # [L, H]

    maskb = np.full((3, 128, W), -1e10, dtype=np.float32)
    j0_last = N - W
    for r in range(128):
        maskb[0, r, max(0, r - HALF):r + HALF + 1] = 0.0               # tile 0 (j0 = 0)
        maskb[1, r, r:r + NB] = 0.0                                    # middle tiles
        i = 128 * (T - 1) + r                                          # last tile
        maskb[2, r, i - HALF - j0_last:min(N - 1, i + HALF) - j0_last + 1] = 0.0

    in_maps = []
    for b in range(B):
        x = input[b, :, 0, :].astype(np.float32)
        mem = state_memory[b, :, 0, :].astype(np.float32)
        in_maps.append({
            "xT": np.ascontiguousarray(x.T),
            "memT": np.ascontiguousarray(mem.T),
            "memN": np.ascontiguousarray(mem * (1.0 - RATE)),
            "wT": wT,
            "scb": scb,
            "maskb": maskb,
        })
    return in_maps


def run(inputs, trace=False):
    from concourse.bass_utils import run_bass_kernel_spmd
    if "nc" not in _cache:
        _cache["nc"] = _build_program()
    nc = _cache["nc"]
    in_maps = _host_prep(**inputs)
    res = run_bass_kernel_spmd(nc, in_maps, core_ids=list(range(B)), trace=trace)
    out = np.empty((B, N, C, L), dtype=np.float32)
    for b in range(B):
        out[b] = res.results[b]["outT"].T.reshape(N, C, L)
    return out, res


def kernel(**inputs):
    out, _ = run(inputs, trace=False)
    return out


# revision 9
# speedup vs baseline: 1.7868x; 1.7868x over previous
"""Trainium2 Bass kernel for MemorySpatialAttention.

Math (per batch b):
  f = LeakyReLU_0.1(BN(conv(x)))  with conv = full-length dot -> x[N,L] @ W[L,H]
  sim = f_in @ f_mem^T  banded to |i-j| <= 8 (17 neighbors, clamped at edges)
  attn = softmax_band(sim);  out = 0.5*x + 0.5*(attn @ mem)

Sharding: data-parallel over batch B=8 -> one batch per NeuronCore, no
collectives.

Per-core layout: features in [H=128, N] (contractions on partitions).
Row tiles of 128 queries use a uniform j-window [128t-8, 128t+136); fmT is
zero-padded by 8 cols on each side so the window never leaves the tensor,
and memNB pre-stages mem rows in window-aligned blocks so attn@mem needs no
per-tile DMA. Softmax runs in [i, j] layout (row-max on the free axis),
E=exp(sim-max) is written bf16, PE-transposed, and attn@mem produces the
natural-layout [i, 56] output where 1/rowsum (from an appended ones column
of memNB) is a per-partition scale.
"""
import sys
sys.path.insert(0, '/opt/trn_rl_repo')

import numpy as np

B, N, C, L, H = 8, 2048, 1, 56, 128
NB, HALF = 17, 8
T = N // 128          # 16 row tiles
W = 128 + 2 * HALF    # 144-wide band window per row tile
LA = L + 1            # mem slice cols + ones col for rowsum
RATE = 0.5
BN_EPS = 1e-5
NEG_SLOPE = 0.1
GRP = 4               # tiles per negmax batch

_cache = {}


def _build_program():
    import concourse.bacc as bacc
    import concourse.tile as tile
    from concourse import mybir
    from concourse.masks import make_identity

    F32 = mybir.dt.float32
    F32R = mybir.dt.float32r
    BF16 = mybir.dt.bfloat16
    AF = mybir.ActivationFunctionType
    AX = mybir.AxisListType

    nc = bacc.Bacc("TRN2", target_bir_lowering=False, debug=False)

    xT = nc.dram_tensor("xT", [L, N], F32R, kind="ExternalInput")
    memT = nc.dram_tensor("memT", [L, N], F32R, kind="ExternalInput")
    memNB = nc.dram_tensor("memNB", [T + 1, 128, LA], BF16, kind="ExternalInput")
    xh = nc.dram_tensor("xh", [N, L], F32, kind="ExternalInput")       # 0.5 * x
    wT = nc.dram_tensor("wT", [L, H], F32R, kind="ExternalInput")
    scb = nc.dram_tensor("scb", [H, 2], F32, kind="ExternalInput")     # BN scale, bias
    maskb = nc.dram_tensor("maskb", [3, 128, W], F32, kind="ExternalInput")
    out = nc.dram_tensor("out", [N, L], F32, kind="ExternalOutput")

    with tile.TileContext(nc) as tc:
        with tc.tile_pool(name="consts", bufs=1) as consts, \
             tc.tile_pool(name="work", bufs=3) as work, \
             tc.tile_pool(name="stats", bufs=2) as stats, \
             tc.tile_pool(name="pbig", bufs=2, space="PSUM") as pbig, \
             tc.tile_pool(name="pt", bufs=3, space="PSUM") as pt_pool, \
             tc.tile_pool(name="pc", bufs=3, space="PSUM") as pc_pool:

            xT_s = consts.tile([L, N], F32R)
            memT_s = consts.tile([L, N], F32R)
            wT_s = consts.tile([L, H], F32R)
            scb_s = consts.tile([H, 2], F32)
            maskb_s = consts.tile([128, 3, W], F32)
            memNB_s = consts.tile([128, T + 1, LA], BF16)
            xh_s = consts.tile([128, T, L], F32)
            ident = consts.tile([128, 128], BF16)
            fiT = consts.tile([H, N], F32)
            fmT = consts.tile([H, N + 2 * HALF], F32)
            simS = consts.tile([128, T, W], F32)
            EB = consts.tile([128, T, W], BF16)
            negmax = consts.tile([128, T], F32)
            outn = consts.tile([128, T, L], F32)

            nc.sync.dma_start(out=wT_s, in_=wT.ap())
            nc.sync.dma_start(out=scb_s, in_=scb.ap())
            nc.sync.dma_start(out=maskb_s, in_=maskb.ap().rearrange("t p w -> p t w"))
            nc.sync.dma_start(out=xT_s, in_=xT.ap())
            nc.sync.dma_start(out=memT_s, in_=memT.ap())
            nc.sync.dma_start(out=memNB_s, in_=memNB.ap().rearrange("t p d -> p t d"))
            nc.sync.dma_start(out=xh_s, in_=xh.ap().rearrange("(t p) d -> p t d", p=128))
            make_identity(nc, ident)
            nc.vector.memset(fmT[:, 0:HALF], 0.0)
            nc.vector.memset(fmT[:, N + HALF:N + 2 * HALF], 0.0)

            # ---- features: f^T = PRelu(BN(W^T @ x^T)) in [H, N] layout ----
            CH = 512
            for c in range(N // CH):
                for src, dst, off in ((xT_s, fiT, 0), (memT_s, fmT, HALF)):
                    psF = pbig.tile([128, CH], F32, tag="pbig")
                    nc.tensor.matmul(psF, lhsT=wT_s, rhs=src[:, c * CH:(c + 1) * CH],
                                     start=True, stop=True)
                    nc.scalar.activation(dst[:, off + c * CH:off + (c + 1) * CH], psF,
                                         AF.Prelu, bias=scb_s[:, 1:2],
                                         scale=scb_s[:, 0:1], alpha=NEG_SLOPE)

            # ---- banded attention, 128-row tiles, grouped stats ----
            for g in range(T // GRP):
                for t in range(g * GRP, (g + 1) * GRP):
                    mtype = 0 if t == 0 else (2 if t == T - 1 else 1)
                    psA = pbig.tile([128, W], F32, tag="pbig")
                    nc.tensor.matmul(psA, lhsT=fiT[:, 128 * t:128 * (t + 1)],
                                     rhs=fmT[:, 128 * t:128 * t + W],
                                     start=True, stop=True)
                    nc.vector.tensor_add(simS[:, t, :], psA, maskb_s[:, mtype, :])
                nc.vector.reduce_max(negmax[:, g * GRP:(g + 1) * GRP],
                                     simS[:, g * GRP:(g + 1) * GRP, :],
                                     axis=AX.X, negate=True)
                for t in range(g * GRP, (g + 1) * GRP):
                    nc.scalar.activation(EB[:, t, :], simS[:, t, :], AF.Exp,
                                         bias=negmax[:, t:t + 1], scale=1.0)

                    psT = pt_pool.tile([128, 256], BF16)
                    nc.tensor.transpose(psT[:, 0:128], EB[:, t, 0:128], ident)
                    nc.tensor.transpose(psT[0:16, 128:256], EB[:, t, 128:W], ident)
                    attnT = work.tile([128, 256], BF16)
                    if t % 2 == 0:
                        nc.scalar.copy(attnT, psT)
                    else:
                        nc.vector.tensor_copy(attnT, psT)

                    psC = pc_pool.tile([128, LA], F32)
                    nc.tensor.matmul(psC, lhsT=attnT[:, 0:128],
                                     rhs=memNB_s[:, t, :], start=True, stop=False)
                    nc.tensor.matmul(psC, lhsT=attnT[0:16, 128:256],
                                     rhs=memNB_s[0:16, t + 1, :], start=False, stop=True)

                    rinv = stats.tile([128, 1], F32)
                    nc.vector.reciprocal(rinv, psC[:, L:LA])
                    tmp = stats.tile([128, L], F32, tag="tmp")
                    nc.scalar.mul(tmp, psC[:, 0:L], rinv)
                    nc.vector.tensor_add(outn[:, t, :], tmp, xh_s[:, t, :])

            for g in range(4):
                nc.sync.dma_start(
                    out=out.ap()[512 * g:512 * (g + 1), :].rearrange(
                        "(t p) d -> p t d", p=128),
                    in_=outn[:, 4 * g:4 * (g + 1), :])

    nc.compile()
    return nc


def _host_prep(input, state_memory, conv_w, conv_b, bn_gamma, bn_beta, bn_mean, bn_var):
    s = (bn_gamma / np.sqrt(bn_var + BN_EPS)).astype(np.float32)
    bias_h = ((conv_b - bn_mean) * s + bn_beta).astype(np.float32)
    scb = np.ascontiguousarray(np.stack([s, bias_h], axis=1))          # [H, 2]
    wT = np.ascontiguousarray(conv_w[:, 0, :].T).astype(np.float32)    # [L, H]

    # window for tile t is global j in [128t-8, 128t+136); local col = j - 128t + 8
    # valid iff |i-j| <= 8 and 0 <= j < N; i = 128t + r -> local cols [r, r+16]
    maskb = np.full((3, 128, W), -1e10, dtype=np.float32)
    for r in range(128):
        maskb[0, r, max(r, HALF):r + NB] = 0.0            # tile 0: j >= 0 -> col >= 8
        maskb[1, r, r:r + NB] = 0.0                       # middle
        maskb[2, r, r:min(r + NB, N - 1 - (128 * (T - 1) - HALF) + 1)] = 0.0  # last

    in_maps = []
    for b in range(B):
        x = np.ascontiguousarray(input[b, :, 0, :]).astype(np.float32)
        mem = np.ascontiguousarray(state_memory[b, :, 0, :]).astype(np.float32)
        # window-aligned mem blocks with ones column: block t = rows [128t-8, 128t+120)
        mnb = np.zeros((T + 1, 128, LA), dtype=np.float32)
        half_mem = (1.0 - RATE) * mem
        for t in range(T + 1):
            lo = 128 * t - HALF
            for_lo = max(0, lo)
            for_hi = min(N, lo + 128)
            if for_lo < for_hi:
                mnb[t, for_lo - lo:for_hi - lo, 0:L] = half_mem[for_lo:for_hi]
                mnb[t, for_lo - lo:for_hi - lo, L] = 1.0
        from ml_dtypes import bfloat16
        in_maps.append({
            "xT": np.ascontiguousarray(x.T),
            "memT": np.ascontiguousarray(mem.T),
            "memNB": mnb.astype(bfloat16),
            "xh": np.ascontiguousarray(RATE * x),
            "wT": wT,
            "scb": scb,
            "maskb": maskb,
        })
    return in_maps


def run(inputs, trace=False):
    from concourse.bass_utils import run_bass_kernel_spmd
    if "nc" not in _cache:
        _cache["nc"] = _build_program()
    nc = _cache["nc"]
    in_maps = _host_prep(**inputs)
    res = run_bass_kernel_spmd(nc, in_maps, core_ids=list(range(B)), trace=trace)
    out = np.empty((B, N, C, L), dtype=np.float32)
    for b in range(B):
        out[b] = res.results[b]["out"].reshape(N, C, L)
    return out, res


def kernel(**inputs):
    out, _ = run(inputs, trace=False)
    return out


# revision 11
# speedup vs baseline: 1.8559x; 1.0387x over previous
"""Trainium2 Bass kernel for MemorySpatialAttention.

Math (per batch b):
  f = LeakyReLU_0.1(BN(conv(x)))  with conv = full-length dot -> x[N,L] @ W[L,H]
  sim = f_in @ f_mem^T  banded to |i-j| <= 8 (17 neighbors, clamped at edges)
  attn = softmax_band(sim);  out = 0.5*x + 0.5*(attn @ mem)

Sharding: data-parallel over batch B=8 -> one batch per NeuronCore, no
collectives.

Per-core structure: features in [H=128, N] layout (L/H contractions on
partitions). Queries are tiled 112 rows at a time with a uniform 128-wide
key window [112t-8, 112t+120) so each tile is exactly one matmul, one PE
transpose and one attn@mem matmul (no K-splits). fmT/fiT are zero-padded so
windows never leave the tensor; band masking is an additive -1e10 constant.
Tiles are processed in groups of 4 sharing one PSUM bank, so the softmax
(mask-add, row-max, subtract, exp, row-sum, reciprocal, normalize, blend)
runs as one batched instruction per group, with per-tile scalars applied
through zero-stride broadcast reads. mem rows are pre-staged window-aligned
(memNB) from a partition-major DRAM image, so no per-tile DMA exists at all.
"""
import sys
sys.path.insert(0, '/opt/trn_rl_repo')

import numpy as np

B, N, C, L, H = 8, 2048, 1, 56, 128
NB, HALF = 17, 8
RT = 112              # query rows per tile
WIN = 128             # key window per tile
T = (N + RT - 1) // RT  # 19 tiles (last partial: 32 rows)
GRP = 4
NG = (T + GRP - 1) // GRP  # 5 groups (last has 3 tiles)
NPAD = RT * T         # 2128
RATE = 0.5
BN_EPS = 1e-5
NEG_SLOPE = 0.1
FI_PAD = NPAD         # fiT cols (2128)
FM_PAD = HALF + N + (RT * (T - 1) + WIN - N)  # 8 + 2048 + 88 = 2144

_cache = {}


def _build_program():
    import concourse.bass as bass
    import concourse.bacc as bacc
    import concourse.tile as tile
    from concourse import mybir
    from concourse.masks import make_identity

    F32 = mybir.dt.float32
    F32R = mybir.dt.float32r
    BF16 = mybir.dt.bfloat16
    AF = mybir.ActivationFunctionType
    AX = mybir.AxisListType

    def bcast(ap_slice, n):
        return bass.AP(tensor=ap_slice.tensor, offset=ap_slice.offset,
                       ap=[*ap_slice.ap, [0, n]])

    nc = bacc.Bacc("TRN2", target_bir_lowering=False, debug=False)

    xT = nc.dram_tensor("xT", [L, N], F32R, kind="ExternalInput")
    memT = nc.dram_tensor("memT", [L, N], F32R, kind="ExternalInput")
    wT = nc.dram_tensor("wT", [L, H], F32R, kind="ExternalInput")
    scb = nc.dram_tensor("scb", [H, 2], F32, kind="ExternalInput")
    maskG = nc.dram_tensor("maskG", [RT, 3 * GRP * WIN], BF16, kind="ExternalInput")
    memNB = nc.dram_tensor("memNB", [128, T * L], BF16, kind="ExternalInput")
    xhp = nc.dram_tensor("xhp", [RT, T * L], F32, kind="ExternalInput")
    out = nc.dram_tensor("out", [RT, T * L], F32, kind="ExternalOutput")

    with tile.TileContext(nc) as tc:
        with tc.tile_pool(name="consts", bufs=1) as consts, \
             tc.tile_pool(name="work", bufs=3) as work, \
             tc.tile_pool(name="pbig", bufs=2, space="PSUM") as pbig, \
             tc.tile_pool(name="pt", bufs=2, space="PSUM") as pt_pool, \
             tc.tile_pool(name="pc", bufs=2, space="PSUM") as pc_pool:

            xT_s = consts.tile([L, N], F32R)
            memT_s = consts.tile([L, N], F32R)
            wT_s = consts.tile([L, H], F32R)
            scb_s = consts.tile([H, 2], F32)
            maskG_s = consts.tile([RT, 3, GRP, WIN], BF16)
            memNB_s = consts.tile([128, T, L], BF16)
            xh_s = consts.tile([RT, T, L], F32)
            ident = consts.tile([RT, RT], BF16)
            fiT = consts.tile([H, FI_PAD], F32)
            fmT = consts.tile([H, FM_PAD], F32)
            simS = consts.tile([RT, T, WIN], F32)
            simB = consts.tile([RT, T, WIN], BF16)
            EB = consts.tile([RT, T, WIN], BF16)
            negmax = consts.tile([RT, T], F32)
            rinv = consts.tile([RT, T], F32)
            outn = consts.tile([RT, T, L], F32)

            nc.sync.dma_start(out=wT_s, in_=wT.ap())
            nc.sync.dma_start(out=scb_s, in_=scb.ap())
            nc.sync.dma_start(out=maskG_s, in_=maskG.ap().rearrange(
                "p (t g w) -> p t g w", g=GRP, w=WIN))
            nc.sync.dma_start(out=memNB_s, in_=memNB.ap().rearrange(
                "p (t d) -> p t d", d=L))
            nc.sync.dma_start(out=xh_s, in_=xhp.ap().rearrange(
                "p (t d) -> p t d", d=L))
            nc.sync.dma_start(out=xT_s, in_=xT.ap())
            nc.sync.dma_start(out=memT_s, in_=memT.ap())
            make_identity(nc, ident)
            nc.vector.memset(fmT[:, 0:HALF], 0.0)
            nc.vector.memset(fmT[:, HALF + N:FM_PAD], 0.0)
            nc.vector.memset(fiT[:, N:FI_PAD], 0.0)

            # ---- features: f^T = PRelu(BN(W^T @ x^T)), fp32r matmuls ----
            CH = 512
            for c in range(2):
                for src, dst, off in ((xT_s, fiT, 0), (memT_s, fmT, HALF)):
                    psF = pbig.tile([128, 2 * CH], F32, tag="pbig")
                    for h in range(2):
                        nc.tensor.matmul(
                            psF[:, h * CH:(h + 1) * CH], lhsT=wT_s,
                            rhs=src[:, (2 * c + h) * CH:(2 * c + h + 1) * CH],
                            start=True, stop=True)
                    nc.scalar.activation(dst[:, off + c * 2 * CH:off + (c + 1) * 2 * CH],
                                         psF, AF.Prelu, bias=scb_s[:, 1:2],
                                         scale=scb_s[:, 0:1], alpha=NEG_SLOPE)

            # ---- banded attention in groups of GRP tiles ----
            for g in range(NG):
                tiles = list(range(g * GRP, min((g + 1) * GRP, T)))
                K = len(tiles)
                gt = 0 if g == 0 else (2 if g == NG - 1 else 1)
                t0 = tiles[0]

                psA = pbig.tile([RT, GRP, WIN], F32, tag="pbig")
                for k, t in enumerate(tiles):
                    nc.tensor.matmul(psA[:, k, :], lhsT=fiT[:, RT * t:RT * (t + 1)],
                                     rhs=fmT[:, RT * t:RT * t + WIN],
                                     start=True, stop=True)

                sS = simS[:, t0:t0 + K, :]
                nc.vector.tensor_add(sS, psA[:, 0:K, :], maskG_s[:, gt, 0:K, :])
                nc.vector.reduce_max(negmax[:, t0:t0 + K], sS, axis=AX.X, negate=True)
                nc.vector.tensor_add(simB[:, t0:t0 + K, :], sS,
                                     bcast(negmax[:, t0:t0 + K], WIN))
                nc.scalar.activation(EB[:, t0:t0 + K, :], simB[:, t0:t0 + K, :], AF.Exp)
                nc.vector.reduce_sum(rinv[:, t0:t0 + K], EB[:, t0:t0 + K, :], axis=AX.X)
                nc.vector.reciprocal(rinv[:, t0:t0 + K], rinv[:, t0:t0 + K])

                psT = pt_pool.tile([128, GRP, RT], BF16)
                for k, t in enumerate(tiles):
                    nc.tensor.transpose(psT[:, k, :], EB[:, t, :], ident)
                attnT = work.tile([128, GRP, RT], BF16)
                nc.scalar.copy(attnT[:, 0:K, :], psT[:, 0:K, :])

                psC = pc_pool.tile([RT, GRP, L], F32)
                for k, t in enumerate(tiles):
                    nc.tensor.matmul(psC[:, k, :], lhsT=attnT[:, k, :],
                                     rhs=memNB_s[:, t, :], start=True, stop=True)

                tmp = work.tile([RT, GRP, L], F32)
                nc.vector.tensor_mul(tmp[:, 0:K, :], psC[:, 0:K, :],
                                     bcast(rinv[:, t0:t0 + K], L))
                nc.vector.tensor_add(outn[:, t0:t0 + K, :], tmp[:, 0:K, :],
                                     xh_s[:, t0:t0 + K, :])

            for h in range(2):
                lo = h * 10
                hi = min(T, lo + 10)
                nc.sync.dma_start(
                    out=out.ap().rearrange("p (t d) -> p t d", d=L)[:, lo:hi, :],
                    in_=outn[:, lo:hi, :])

    nc.compile()
    return nc


def _host_prep(input, state_memory, conv_w, conv_b, bn_gamma, bn_beta, bn_mean, bn_var):
    from ml_dtypes import bfloat16

    s = (bn_gamma / np.sqrt(bn_var + BN_EPS)).astype(np.float32)
    bias_h = ((conv_b - bn_mean) * s + bn_beta).astype(np.float32)
    scb = np.ascontiguousarray(np.stack([s, bias_h], axis=1))          # [H, 2]
    wT = np.ascontiguousarray(conv_w[:, 0, :].T).astype(np.float32)    # [L, H]

    # Per-tile mask [RT, WIN]: tile t covers queries i = RT*t + r, keys
    # j = RT*t - 8 + c  (c = local col). Band |i-j| <= 8 -> c in [r, r+16],
    # clipped by 0 <= j < N and i < N.
    def tile_mask(t):
        m = np.full((RT, WIN), -1e10, dtype=np.float32)
        for r in range(RT):
            i = RT * t + r
            if i >= N:
                continue
            lo = max(i - HALF, 0) - (RT * t - HALF)
            hi = min(i + HALF, N - 1) - (RT * t - HALF)
            m[r, lo:hi + 1] = 0.0
        return m

    mids = tile_mask(1)
    maskG = np.empty((3, GRP, RT, WIN), dtype=np.float32)
    maskG[:] = mids[None, None]
    maskG[0, 0] = tile_mask(0)
    maskG[2, T - 1 - (NG - 1) * GRP] = tile_mask(T - 1)
    maskG = np.ascontiguousarray(maskG.transpose(2, 0, 1, 3).reshape(RT, -1))

    in_maps = []
    for b in range(B):
        x = np.ascontiguousarray(input[b, :, 0, :]).astype(np.float32)
        mem = np.ascontiguousarray(state_memory[b, :, 0, :]).astype(np.float32)
        # window-aligned mem blocks: block t = rows [RT*t-8, RT*t+120)
        mnb = np.zeros((T, 128, L), dtype=np.float32)
        half_mem = (1.0 - RATE) * mem
        for t in range(T):
            lo = RT * t - HALF
            a, bnd = max(0, lo), min(N, lo + 128)
            if a < bnd:
                mnb[t, a - lo:bnd - lo] = half_mem[a:bnd]
        xh = np.zeros((T, RT, L), dtype=np.float32)
        xh.reshape(-1, L)[:N] = RATE * x
        in_maps.append({
            "xT": np.ascontiguousarray(x.T),
            "memT": np.ascontiguousarray(mem.T),
            "wT": wT,
            "scb": scb,
            "maskG": maskG.astype(bfloat16),
            "memNB": np.ascontiguousarray(
                mnb.transpose(1, 0, 2).reshape(128, -1)).astype(bfloat16),
            "xhp": np.ascontiguousarray(xh.transpose(1, 0, 2).reshape(RT, -1)),
        })
    return in_maps


def run(inputs, trace=False):
    from concourse.bass_utils import run_bass_kernel_spmd
    if "nc" not in _cache:
        _cache["nc"] = _build_program()
    nc = _cache["nc"]
    in_maps = _host_prep(**inputs)
    res = run_bass_kernel_spmd(nc, in_maps, core_ids=list(range(B)), trace=trace)
    out = np.empty((B, N, C, L), dtype=np.float32)
    for b in range(B):
        o = res.results[b]["out"].reshape(RT, T, L).transpose(1, 0, 2)
        out[b] = o.reshape(NPAD, L)[:N].reshape(N, C, L)
    return out, res


def kernel(**inputs):
    out, _ = run(inputs, trace=False)
    return out


# revision 17
# speedup vs baseline: 1.8704x; 1.0078x over previous
"""Trainium2 Bass kernel for MemorySpatialAttention.

Math (per batch b):
  f = LeakyReLU_0.1(BN(conv(x)))  with conv = full-length dot -> x[N,L] @ W[L,H]
  sim = f_in @ f_mem^T  banded to |i-j| <= 8 (17 neighbors, clamped at edges)
  attn = softmax_band(sim);  out = 0.5*x + 0.5*(attn @ mem)

Sharding: data-parallel over batch B=8 -> one batch per NeuronCore, no
collectives.

Per-core structure: features in [H=128, N] layout (L/H contractions on
partitions). Queries are tiled 112 rows at a time with a uniform 128-wide
key window [112t-8, 112t+120) so each tile is exactly one matmul, one PE
transpose and one attn@mem matmul (no K-splits). fmT/fiT are zero-padded so
windows never leave the tensor; band masking is an additive -1e10 constant.
Tiles are processed in groups of 4 sharing one PSUM bank, so the softmax
(mask-add, row-max, subtract, exp, row-sum, reciprocal, normalize, blend)
runs as one batched instruction per group, with per-tile scalars applied
through zero-stride broadcast reads. mem rows are pre-staged window-aligned
(memNB) from a partition-major DRAM image, so no per-tile DMA exists at all.
"""
import sys
sys.path.insert(0, '/opt/trn_rl_repo')

import numpy as np

B, N, C, L, H = 8, 2048, 1, 56, 128
NB, HALF = 17, 8
RT = 112              # query rows per tile
WIN = 128             # key window per tile
T = (N + RT - 1) // RT  # 19 tiles (last partial: 32 rows)
GRP = 4
NG = (T + GRP - 1) // GRP  # 5 groups (last has 3 tiles)
NPAD = RT * T         # 2128
RATE = 0.5
BN_EPS = 1e-5
NEG_SLOPE = 0.1
FI_PAD = NPAD         # fiT cols (2128)
FM_PAD = HALF + N + (RT * (T - 1) + WIN - N)  # 8 + 2048 + 88 = 2144

_cache = {}


def _build_program():
    import concourse.bass as bass
    import concourse.bacc as bacc
    import concourse.tile as tile
    from concourse import mybir
    from concourse.masks import make_identity

    F32 = mybir.dt.float32
    F32R = mybir.dt.float32r
    BF16 = mybir.dt.bfloat16
    AF = mybir.ActivationFunctionType
    AX = mybir.AxisListType

    def bcast(ap_slice, n):
        return bass.AP(tensor=ap_slice.tensor, offset=ap_slice.offset,
                       ap=[*ap_slice.ap, [0, n]])

    nc = bacc.Bacc("TRN2", target_bir_lowering=False, debug=False)

    xT = nc.dram_tensor("xT", [L, N], F32R, kind="ExternalInput")
    memT = nc.dram_tensor("memT", [L, N], F32R, kind="ExternalInput")
    wT = nc.dram_tensor("wT", [L, H], F32R, kind="ExternalInput")
    scb = nc.dram_tensor("scb", [H, 2], F32, kind="ExternalInput")
    maskG = nc.dram_tensor("maskG", [RT, 3 * GRP * WIN], BF16, kind="ExternalInput")
    memNB = nc.dram_tensor("memNB", [128, T * L], BF16, kind="ExternalInput")
    xhp = nc.dram_tensor("xhp", [RT, T * L], F32, kind="ExternalInput")
    out = nc.dram_tensor("out", [RT, T * L], F32, kind="ExternalOutput")

    with tile.TileContext(nc) as tc:
        with tc.tile_pool(name="consts", bufs=1) as consts, \
             tc.tile_pool(name="work", bufs=3) as work, \
             tc.tile_pool(name="pbig", bufs=2, space="PSUM") as pbig, \
             tc.tile_pool(name="pt", bufs=2, space="PSUM") as pt_pool, \
             tc.tile_pool(name="pc", bufs=2, space="PSUM") as pc_pool:

            xT_s = consts.tile([L, N], F32R)
            memT_s = consts.tile([L, N], F32R)
            wT_s = consts.tile([L, H], F32R)
            scb_s = consts.tile([H, 2], F32)
            maskG_s = consts.tile([RT, 3, GRP, WIN], BF16)
            memNB_s = consts.tile([128, T, L], BF16)
            xh_s = consts.tile([RT, T, L], F32)
            ident = consts.tile([RT, RT], BF16)
            fiT = consts.tile([H, FI_PAD], F32)
            fmT = consts.tile([H, FM_PAD], F32)
            simS = consts.tile([RT, T, WIN], F32)
            simB = consts.tile([RT, T, WIN], BF16)
            EB = consts.tile([RT, T, WIN], BF16)
            negmax = consts.tile([RT, T], F32)
            rinv = consts.tile([RT, T], F32)
            outn = consts.tile([RT, T, L], F32)

            # gating DMAs first (features need wT/xT/memT); bulk consts on
            # other queues so one DGE ring doesn't serialize the transfers
            nc.sync.dma_start(out=wT_s, in_=wT.ap())
            nc.sync.dma_start(out=xT_s, in_=xT.ap())
            nc.sync.dma_start(out=memT_s, in_=memT.ap())
            nc.sync.dma_start(out=scb_s, in_=scb.ap())
            nc.gpsimd.dma_start(out=maskG_s, in_=maskG.ap().rearrange(
                "p (t g w) -> p t g w", g=GRP, w=WIN))
            nc.scalar.dma_start(out=memNB_s, in_=memNB.ap().rearrange(
                "p (t d) -> p t d", d=L))
            nc.gpsimd.dma_start(out=xh_s, in_=xhp.ap().rearrange(
                "p (t d) -> p t d", d=L))
            make_identity(nc, ident)
            nc.vector.memset(fmT[:, 0:HALF], 0.0)
            nc.vector.memset(fmT[:, HALF + N:FM_PAD], 0.0)
            nc.vector.memset(fiT[:, N:FI_PAD], 0.0)

            # ---- features: f^T = PRelu(BN(W^T @ x^T)), fp32r matmuls ----
            CH = 512
            for c in range(2):
                for src, dst, off in ((xT_s, fiT, 0), (memT_s, fmT, HALF)):
                    psF = pbig.tile([128, 2 * CH], F32, tag="pbig", name="psF")
                    for h in range(2):
                        nc.tensor.matmul(
                            psF[:, h * CH:(h + 1) * CH], lhsT=wT_s,
                            rhs=src[:, (2 * c + h) * CH:(2 * c + h + 1) * CH],
                            start=True, stop=True)
                    nc.scalar.activation(dst[:, off + c * 2 * CH:off + (c + 1) * 2 * CH],
                                         psF, AF.Prelu, bias=scb_s[:, 1:2],
                                         scale=scb_s[:, 0:1], alpha=NEG_SLOPE)

            # ---- banded attention in groups of GRP tiles ----
            for g in range(NG):
                tiles = list(range(g * GRP, min((g + 1) * GRP, T)))
                K = len(tiles)
                gt = 0 if g == 0 else (2 if g == NG - 1 else 1)
                t0 = tiles[0]

                psA = pbig.tile([RT, GRP, WIN], F32, tag="pbig", name="psA")
                for k, t in enumerate(tiles):
                    nc.tensor.matmul(psA[:, k, :], lhsT=fiT[:, RT * t:RT * (t + 1)],
                                     rhs=fmT[:, RT * t:RT * t + WIN],
                                     start=True, stop=True)

                sS = simS[:, t0:t0 + K, :]
                nc.vector.tensor_add(sS, psA[:, 0:K, :], maskG_s[:, gt, 0:K, :])
                nc.vector.reduce_max(negmax[:, t0:t0 + K], sS, axis=AX.X, negate=True)
                nc.vector.tensor_add(simB[:, t0:t0 + K, :], sS,
                                     bcast(negmax[:, t0:t0 + K], WIN))
                nc.scalar.activation(EB[:, t0:t0 + K, :], simB[:, t0:t0 + K, :], AF.Exp)
                nc.vector.reduce_sum(rinv[:, t0:t0 + K], EB[:, t0:t0 + K, :], axis=AX.X)
                nc.vector.reciprocal(rinv[:, t0:t0 + K], rinv[:, t0:t0 + K])

                psT = pt_pool.tile([128, GRP, RT], BF16)
                for k, t in enumerate(tiles):
                    nc.tensor.transpose(psT[:, k, :], EB[:, t, :], ident)
                attnT = work.tile([128, GRP, RT], BF16)
                if g % 2 == 0:
                    nc.scalar.copy(attnT[:, 0:K, :], psT[:, 0:K, :])
                else:
                    nc.vector.tensor_copy(attnT[:, 0:K, :], psT[:, 0:K, :])

                psC = pc_pool.tile([RT, GRP, L], F32)
                for k, t in enumerate(tiles):
                    nc.tensor.matmul(psC[:, k, :], lhsT=attnT[:, k, :],
                                     rhs=memNB_s[:, t, :], start=True, stop=True)

                tmp = work.tile([RT, GRP, L], F32)
                nc.vector.tensor_mul(tmp[:, 0:K, :], psC[:, 0:K, :],
                                     bcast(rinv[:, t0:t0 + K], L))
                nc.vector.tensor_add(outn[:, t0:t0 + K, :], tmp[:, 0:K, :],
                                     xh_s[:, t0:t0 + K, :])

            for h in range(2):
                lo = h * 10
                hi = min(T, lo + 10)
                nc.sync.dma_start(
                    out=out.ap().rearrange("p (t d) -> p t d", d=L)[:, lo:hi, :],
                    in_=outn[:, lo:hi, :])

    nc.compile()
    return nc


def _host_prep(input, state_memory, conv_w, conv_b, bn_gamma, bn_beta, bn_mean, bn_var):
    from ml_dtypes import bfloat16

    s = (bn_gamma / np.sqrt(bn_var + BN_EPS)).astype(np.float32)
    bias_h = ((conv_b - bn_mean) * s + bn_beta).astype(np.float32)
    scb = np.ascontiguousarray(np.stack([s, bias_h], axis=1))          # [H, 2]
    wT = np.ascontiguousarray(conv_w[:, 0, :].T).astype(np.float32)    # [L, H]

    # Per-tile mask [RT, WIN]: tile t covers queries i = RT*t + r, keys
    # j = RT*t - 8 + c  (c = local col). Band |i-j| <= 8 -> c in [r, r+16],
    # clipped by 0 <= j < N and i < N.
    def tile_mask(t):
        m = np.full((RT, WIN), -1e10, dtype=np.float32)
        for r in range(RT):
            i = RT * t + r
            if i >= N:
                continue
            lo = max(i - HALF, 0) - (RT * t - HALF)
            hi = min(i + HALF, N - 1) - (RT * t - HALF)
            m[r, lo:hi + 1] = 0.0
        return m

    mids = tile_mask(1)
    maskG = np.empty((3, GRP, RT, WIN), dtype=np.float32)
    maskG[:] = mids[None, None]
    maskG[0, 0] = tile_mask(0)
    maskG[2, T - 1 - (NG - 1) * GRP] = tile_mask(T - 1)
    maskG = np.ascontiguousarray(maskG.transpose(2, 0, 1, 3).reshape(RT, -1))

    in_maps = []
    for b in range(B):
        x = np.ascontiguousarray(input[b, :, 0, :]).astype(np.float32)
        mem = np.ascontiguousarray(state_memory[b, :, 0, :]).astype(np.float32)
        # window-aligned mem blocks: block t = rows [RT*t-8, RT*t+120)
        mnb = np.zeros((T, 128, L), dtype=np.float32)
        half_mem = (1.0 - RATE) * mem
        for t in range(T):
            lo = RT * t - HALF
            a, bnd = max(0, lo), min(N, lo + 128)
            if a < bnd:
                mnb[t, a - lo:bnd - lo] = half_mem[a:bnd]
        xh = np.zeros((T, RT, L), dtype=np.float32)
        xh.reshape(-1, L)[:N] = RATE * x
        in_maps.append({
            "xT": np.ascontiguousarray(x.T),
            "memT": np.ascontiguousarray(mem.T),
            "wT": wT,
            "scb": scb,
            "maskG": maskG.astype(bfloat16),
            "memNB": np.ascontiguousarray(
                mnb.transpose(1, 0, 2).reshape(128, -1)).astype(bfloat16),
            "xhp": np.ascontiguousarray(xh.transpose(1, 0, 2).reshape(RT, -1)),
        })
    return in_maps


def run(inputs, trace=False):
    from concourse.bass_utils import run_bass_kernel_spmd
    if "nc" not in _cache:
        _cache["nc"] = _build_program()
    nc = _cache["nc"]
    in_maps = _host_prep(**inputs)
    res = run_bass_kernel_spmd(nc, in_maps, core_ids=list(range(B)), trace=trace)
    out = np.empty((B, N, C, L), dtype=np.float32)
    for b in range(B):
        o = res.results[b]["out"].reshape(RT, T, L).transpose(1, 0, 2)
        out[b] = o.reshape(NPAD, L)[:N].reshape(N, C, L)
    return out, res


def kernel(**inputs):
    out, _ = run(inputs, trace=False)
    return out
